# revision 14
# baseline (speedup 1.0000x reference)
"""Distributed Trainium2 kernel for pre-LN multi-head self-attention.

Reference computation (b=1, n=4096, d=1024, h=16, hd=64):
    x  = layernorm(q, gamma, beta)
    qp = x @ Wq.T ; kv = x @ Wkv.T ; k, v = kv[:, :d], kv[:, d:]
    per head: out_h = softmax(qh @ kh.T / sqrt(hd)) @ vh
    out = concat_heads @ Wo.T

Sharding (8 cores): head-parallel. Core c owns heads 2c, 2c+1 (128
columns of qp/k/v). Each core gets q transposed in bf16, its 128-row
slice of Wq/Wk/Wv (pre-transposed, gamma folded in) plus those slices'
row sums, and the full Wo.T. After attention, per-pass AllGathers swap
head-shards for seq-shards; each core then computes final rows
[c*512, (c+1)*512) with the full Wo, locating its shard with a
partition-id-indexed dynamic DMA.

Layernorm is never materialized: with x = (q - mu) * rstd (gamma folded
into W, beta = 0),
    (W @ x.T)[e, s] = rstd[s] * ((W @ q.T)[e, s] - wsum[e] * mu[s])
so each projection runs on raw q.T plus one K=1 rank-1 correction
matmul (lhsT = wsum, rhs = -mu), and the rstd factors are folded where
they are cheapest: into the Q projection's psum->sbuf multiply (rstd
broadcast across partitions via a DRAM round-trip DMA), into the V
copy-out (per-partition tensor_scalar), and for the K side into the
exp's per-partition scale AP (together with the 1/sqrt(hd) softmax
scale) - the K projection itself is a plain copy.

Attention is computed transposed (S.T [keys, queries]) so the exp'd
probabilities feed the P@V matmul directly as the moving operand - no
PE transposes anywhere. The two heads' S matmuls share the PE array via
row-split tile positions (base partitions 0/64, K=64 each). The softmax
denominator comes free from a ones-column appended to V; the final
division broadcasts 1/rowsum with another DRAM round-trip DMA.
"""

import sys

sys.path.insert(0, "/opt/trn_rl_repo")

from contextlib import ExitStack

import ml_dtypes
import numpy as np

import concourse.bass as bass
import concourse.tile as tile
from concourse import bacc, mybir
from concourse.bass_utils import run_bass_kernel_spmd

F32 = mybir.dt.float32
BF16 = mybir.dt.bfloat16
U32 = mybir.dt.uint32
EXP = mybir.ActivationFunctionType.Exp
LOG = mybir.ActivationFunctionType.Ln

NC_ = 8  # cores
D = 1024  # hidden
N = 4096  # sequence
H = 16  # heads
HD = 64  # head dim
EL = (H // NC_) * HD  # local embedding width per core = 128
SC = N // NC_  # output seq rows per core = 512
DCH = D // 128  # d chunks of 128 = 8
SCH = N // 512  # s chunks of 512 = 8
KB = N // 128  # key blocks of 128 = 32
NPASS = 4  # attention passes, 2 q-chunks each
EPS = 1e-5
SCALE = HD**-0.5

_compiled = {}


def _build(phases="ABCX"):
    nc = bacc.Bacc("TRN2", target_bir_lowering=False, debug=False, num_devices=NC_)

    qT = nc.declare_dram_parameter("qT", [D, N], BF16, isOutput=False)
    wq = nc.declare_dram_parameter("wq", [D, EL], BF16, isOutput=False)
    wk = nc.declare_dram_parameter("wk", [D, EL], BF16, isOutput=False)
    wv = nc.declare_dram_parameter("wv", [D, EL], BF16, isOutput=False)
    wo = nc.declare_dram_parameter("wo", [D, D], BF16, isOutput=False)  # Wo.T
    wsq = nc.declare_dram_parameter("wsq", [1, EL], BF16, isOutput=False)
    wsk = nc.declare_dram_parameter("wsk", [1, EL], BF16, isOutput=False)
    wsv = nc.declare_dram_parameter("wsv", [1, EL], BF16, isOutput=False)
    agp = nc.declare_dram_parameter("agp", [1, 1], U32, isOutput=False)  # c // 2
    agr = nc.declare_dram_parameter("agr", [1, 1], U32, isOutput=False)  # c%2*512
    out_ext = nc.declare_dram_parameter("out", [SC, D], F32, isOutput=True)

    ag_in = nc.dram_tensor("ag_in", [NPASS, EL, 2 * SC], BF16)
    ag_out = nc.dram_tensor("ag_out", [NPASS, NC_, EL, 2 * SC], BF16)
    rstd_dram = nc.dram_tensor("rstd_dram", [1, N], F32)
    rec_dram = nc.dram_tensor("rec_dram", [16, SC], F32)

    with tile.TileContext(nc) as tc, ExitStack() as top:
        const_pool = top.enter_context(tc.tile_pool(name="const", bufs=1))
        ones_col = const_pool.tile([128, 1], BF16)  # stats matmul lhsT
        nc.vector.memset(ones_col[:], 1.0)

        wts = top.enter_context(tc.tile_pool(name="wts", bufs=1))
        wq_sb = wts.tile([128, DCH * EL], BF16, tag="wq")
        wk_sb = wts.tile([128, DCH * EL], BF16, tag="wk")
        wv_sb = wts.tile([128, DCH * EL], BF16, tag="wv")
        wo_sb = wts.tile([128, DCH * D], BF16, tag="wo")
        for t, src in ((wq_sb, wq), (wk_sb, wk), (wv_sb, wv)):
            nc.sync.dma_start(
                t[:].rearrange("p (c e) -> p c e", e=EL),
                src.ap().rearrange("(c p) e -> p c e", p=128),
            )
        nc.sync.dma_start(
            wo_sb[:].rearrange("p (c o) -> p c o", o=D),
            wo.ap().rearrange("(c p) o -> p c o", p=128),
        )
        wsq_sb = wts.tile([1, EL], BF16, tag="wsq")
        wsk_sb = wts.tile([1, EL], BF16, tag="wsk")
        wsv_sb = wts.tile([1, EL], BF16, tag="wsv")
        for t, src in ((wsq_sb, wsq), (wsk_sb, wsk), (wsv_sb, wsv)):
            nc.sync.dma_start(t[:], src.ap())

        proj = top.enter_context(tc.tile_pool(name="proj", bufs=1))
        qpT_sb = proj.tile([128, N], BF16, tag="qpT")  # [e_local, s], rstd applied
        kpT_sb = proj.tile([128, N], BF16, tag="kpT")  # [e_local, s], raw-corrected
        vp_sb = proj.tile([128, KB * 130], BF16, tag="vp")  # per kb: v_h0|1|v_h1|1
        obuf = proj.tile([128, N], BF16, tag="obuf")  # O.T [e_local, s]
        rstdT = proj.tile([128, KB], F32, tag="rstdT")  # rstd, key-major layout
        rstdT8 = proj.tile([128, KB], F32, tag="rstdT8")  # rstd * 1/sqrt(hd)

        vre = vp_sb[:].rearrange("p (kb c) -> p kb c", c=130)
        nc.gpsimd.memset(vre[:, :, 64:65], 1.0)
        nc.gpsimd.memset(vre[:, :, 129:130], 1.0)

        # ---- Phase A: LN stats + LN-folded projections, per 512-col s-chunk
        with ExitStack() as pa:
            qt_pool = pa.enter_context(tc.tile_pool(name="qt", bufs=20))
            sq_pool = pa.enter_context(tc.tile_pool(name="sq", bufs=3))
            st_pool = pa.enter_context(tc.tile_pool(name="st", bufs=2))
            rb_pool = pa.enter_context(tc.tile_pool(name="rb", bufs=2))
            ps_stat = pa.enter_context(tc.tile_pool(name="ps_stat", bufs=1, space="PSUM"))
            ps_proj = pa.enter_context(tc.tile_pool(name="ps_proj", bufs=3, space="PSUM"))
            ps_v = pa.enter_context(tc.tile_pool(name="ps_v", bufs=3, space="PSUM"))

            for sc in range(SCH):
                ssl = bass.ts(sc, 512)
                qts = []
                for dc in range(DCH):
                    t = qt_pool.tile([128, 512], BF16, tag="qt")
                    nc.sync.dma_start(t[:], qT[dc * 128 : (dc + 1) * 128, ssl])
                    qts.append(t)
                # stats: column sums of qT and qT^2 via ones-matmuls
                psum = ps_stat.tile([1, 512], F32, tag="sum")
                psq = ps_stat.tile([1, 512], F32, tag="sumsq")
                for dc in range(DCH):
                    nc.tensor.matmul(
                        psum[:], ones_col[:], qts[dc][:],
                        start=(dc == 0), stop=(dc == DCH - 1),
                    )
                for dc in range(DCH):
                    sq = sq_pool.tile([128, 512], BF16, tag="sq")
                    nc.vector.tensor_mul(sq[:], qts[dc][:], qts[dc][:])
                    nc.tensor.matmul(
                        psq[:], ones_col[:], sq[:],
                        start=(dc == 0), stop=(dc == DCH - 1),
                    )
                mu = st_pool.tile([1, 512], F32, tag="mu")
                nc.vector.tensor_scalar_mul(mu[:], psum[:], 1.0 / D)
                negmu = st_pool.tile([1, 512], BF16, tag="negmu")
                nc.vector.tensor_scalar_mul(negmu[:], psum[:], -1.0 / D)
                msq = st_pool.tile([1, 512], F32, tag="msq")
                nc.vector.tensor_scalar_mul(msq[:], psq[:], 1.0 / D)
                mu2 = st_pool.tile([1, 512], F32, tag="mu2")
                nc.vector.tensor_mul(mu2[:], mu[:], mu[:])
                var = st_pool.tile([1, 512], F32, tag="var")
                nc.vector.scalar_tensor_tensor(
                    var[:], msq[:], EPS, mu2[:],
                    op0=mybir.AluOpType.add, op1=mybir.AluOpType.subtract,
                )
                # rstd = exp(-0.5 * log(var + eps)) — keeps ACT on one table set
                logv = st_pool.tile([1, 512], F32, tag="logv")
                nc.scalar.activation(logv[:], var[:], LOG)
                rstd = st_pool.tile([1, 512], F32, tag="rstd")
                nc.scalar.activation(rstd[:], logv[:], EXP, scale=-0.5)
                # distribute rstd via DRAM: partition-broadcast + key-major
                nc.sync.dma_start(rstd_dram[0:1, ssl], rstd[:])
                rstd_b = rb_pool.tile([128, 512], F32, tag="rstd_b")
                nc.sync.dma_start(
                    rstd_b[:], rstd_dram.ap()[0:1, ssl].to_broadcast((128, 512))
                )
                nc.sync.dma_start(
                    rstdT[:, sc * 4 : (sc + 1) * 4],
                    rstd_dram.ap()[0:1, ssl].rearrange("o (c p) -> (o p) c", p=128),
                )
                # q/k projections: psum[e, s-chunk] over d-chunks + rank-1 fix
                for dst, w, ws, scale_q in (
                    (qpT_sb, wq_sb, wsq_sb, True),
                    (kpT_sb, wk_sb, wsk_sb, False),
                ):
                    pp = ps_proj.tile([128, 512], F32, tag="pp", name=f"pp{sc}_{scale_q}")
                    for dc in range(DCH):
                        nc.tensor.matmul(
                            pp[:], w[:, bass.ts(dc, EL)], qts[dc][:],
                            start=(dc == 0), stop=False,
                        )
                    nc.tensor.matmul(pp[:], ws[:], negmu[:], start=False, stop=True)
                    if scale_q:
                        nc.vector.tensor_mul(dst[:, ssl], pp[:], rstd_b[:])
                    else:
                        nc.vector.tensor_copy(dst[:, ssl], pp[:])
                # v projection: [s, e] layout per 128-row s-tile + rank-1 fix
                for st in range(4):
                    stg = sc * 4 + st
                    pv = ps_v.tile([128, EL], F32, tag="pv", name=f"pv{stg}")
                    for dc in range(DCH):
                        nc.tensor.matmul(
                            pv[:], qts[dc][:, bass.ts(st, 128)],
                            wv_sb[:, bass.ts(dc, EL)],
                            start=(dc == 0), stop=False,
                        )
                    nc.tensor.matmul(
                        pv[:], negmu[0:1, bass.ts(st, 128)], wsv_sb[:],
                        start=False, stop=True,
                    )
                    base = stg * 130
                    nc.vector.tensor_scalar_mul(
                        vp_sb[:, base : base + 64], pv[:, 0:64],
                        rstdT[:, stg : stg + 1],
                    )
                    nc.vector.tensor_scalar_mul(
                        vp_sb[:, base + 65 : base + 129], pv[:, 64:128],
                        rstdT[:, stg : stg + 1],
                    )
            nc.vector.tensor_scalar_mul(rstdT8[:], rstdT[:], SCALE)

        # ---- Phase B: attention, heads staggered, 2 q-chunks per pass ----
        with ExitStack() as pb:
          if "B" in phases:
            pt_pool = pb.enter_context(tc.tile_pool(name="pt", bufs=4))
            rec_pool = pb.enter_context(tc.tile_pool(name="rec", bufs=2))
            ps_s = pb.enter_context(tc.tile_pool(name="ps_s", bufs=1, space="PSUM"))
            ps_o = pb.enter_context(tc.tile_pool(name="ps_o", bufs=4, space="PSUM"))

            for pas in range(NPASS):
                psl = bass.ts(pas, 1024)
                po = [
                    [
                        ps_o.tile([65, 512], F32, tag="po", name=f"po{pas}_{h}_{qi}")
                        for qi in range(2)
                    ]
                    for h in range(2)
                ]
                for kb in range(KB):
                    pss = []
                    for h in range(2):
                        hs = slice(h * 64, (h + 1) * 64)
                        ps = ps_s.tile(
                            [128, 1024], F32, tag=f"s{h}", name=f"s{pas}_{kb}_{h}"
                        )
                        for qi in range(2):
                            nc.tensor.matmul(
                                ps[:, bass.ts(qi, 512)],
                                kpT_sb[hs, bass.ts(kb, 128)],
                                qpT_sb[hs, bass.ts(2 * pas + qi, 512)],
                                start=True, stop=True,
                            )
                        pss.append(ps)
                    for h in range(2):
                        pt = pt_pool.tile([128, 1024], BF16, tag="pt")
                        nc.scalar.activation(
                            pt[:], pss[h][:], EXP, scale=rstdT8[:, kb : kb + 1]
                        )
                        for qi in range(2):
                            nc.tensor.matmul(
                                po[h][qi][:],
                                vp_sb[:, kb * 130 + h * 65 : kb * 130 + (h + 1) * 65],
                                pt[:, bass.ts(qi, 512)],
                                start=(kb == 0), stop=(kb == KB - 1),
                            )
                # epilogue: divide by the accumulated row sums (psum row 64)
                for h in range(2):
                    for qi in range(2):
                        qc = 2 * pas + qi
                        ridx = pas * 4 + h * 2 + qi
                        rec = rec_pool.tile([1, 512], F32, tag="rec")
                        nc.vector.reciprocal(rec[:], po[h][qi][64:65, :])
                        nc.sync.dma_start(rec_dram[ridx : ridx + 1, :], rec[:])
                        recb = rec_pool.tile([64, 512], F32, tag="recb")
                        nc.sync.dma_start(
                            recb[:],
                            rec_dram.ap()[ridx : ridx + 1, :].to_broadcast((64, 512)),
                        )
                        nc.vector.tensor_mul(
                            obuf[h * 64 : (h + 1) * 64, bass.ts(qc, 512)],
                            po[h][qi][0:64, :], recb[:],
                        )
                nc.sync.dma_start(ag_in[pas], obuf[:, psl])
                if "X" in phases:
                    nc.gpsimd.collective_compute(
                        "AllGather",
                        mybir.AluOpType.bypass,
                        replica_groups=[list(range(NC_))],
                        ins=[ag_in[pas].opt()],
                        outs=[ag_out[pas].opt()],
                    )

        # ---- Phase C: pick this core's seq shard, apply Wo ----
        with ExitStack() as pc:
          if "C" in phases:
            og_pool = pc.enter_context(tc.tile_pool(name="og", bufs=1))
            fo_pool = pc.enter_context(tc.tile_pool(name="fo", bufs=3))
            ps_f = pc.enter_context(tc.tile_pool(name="ps_f", bufs=2, space="PSUM"))

            r1 = nc.sync.alloc_register("agp_r")
            nc.sync.reg_load(r1, agp[0:1, 0:1])
            v1 = nc.sync.snap(r1, donate=True, min_val=0, max_val=NPASS - 1)
            r2 = nc.sync.alloc_register("agr_r")
            nc.sync.reg_load(r2, agr[0:1, 0:1])
            v2 = nc.sync.snap(r2, donate=True, min_val=0, max_val=SC)

            og = og_pool.tile([128, NC_ * SC], BF16, tag="og")  # [e_chunk, i*s]
            for i in range(NC_):
                nc.sync.dma_start(
                    og[:, bass.ts(i, SC)],
                    ag_out.ap()[bass.ds(v1, 1), i, :, bass.ds(v2, SC)],
                )
            for st in range(SC // 128):
                for oc in range(2):
                    pf = ps_f.tile([128, 512], F32, tag="pf")
                    for i in range(NC_):
                        nc.tensor.matmul(
                            pf[:],
                            og[:, i * SC + st * 128 : i * SC + (st + 1) * 128],
                            wo_sb[:, i * D + oc * 512 : i * D + (oc + 1) * 512],
                            start=(i == 0), stop=(i == NC_ - 1),
                        )
                    fo = fo_pool.tile([128, 512], F32, tag="fo", name=f"fo{st}_{oc}")
                    nc.vector.tensor_copy(fo[:], pf[:])
                    nc.sync.dma_start(
                        out_ext[st * 128 : (st + 1) * 128, bass.ts(oc, 512)], fo[:]
                    )

    nc.compile()
    return nc


def _prep_inputs(q, gamma, beta, Wq, Wkv, Wo):
    q = np.asarray(q, dtype=np.float32)
    gamma = np.asarray(gamma, dtype=np.float32)
    beta = np.asarray(beta, dtype=np.float32)
    Wq = np.asarray(Wq, dtype=np.float32)
    Wkv = np.asarray(Wkv, dtype=np.float32)
    Wo = np.asarray(Wo, dtype=np.float32)
    assert not np.any(beta), "nonzero beta not supported by this kernel build"

    bf = ml_dtypes.bfloat16
    qT = np.ascontiguousarray(q.reshape(N, D).T).astype(bf)  # [d, n]
    wq_eff = Wq * gamma[None, :]
    wk_eff = Wkv[:D] * gamma[None, :]
    wv_eff = Wkv[D:] * gamma[None, :]
    woT = np.ascontiguousarray(Wo.T).astype(bf)  # [e, o]

    in_maps = []
    for c in range(NC_):
        rs = slice(c * EL, (c + 1) * EL)
        in_maps.append(
            {
                "qT": qT,
                "wq": np.ascontiguousarray(wq_eff[rs].T).astype(bf),
                "wk": np.ascontiguousarray(wk_eff[rs].T).astype(bf),
                "wv": np.ascontiguousarray(wv_eff[rs].T).astype(bf),
                "wo": woT,
                "wsq": wq_eff[rs].sum(axis=1)[None, :].astype(bf),
                "wsk": wk_eff[rs].sum(axis=1)[None, :].astype(bf),
                "wsv": wv_eff[rs].sum(axis=1)[None, :].astype(bf),
                "agp": np.array([[c // 2]], np.uint32),
                "agr": np.array([[(c % 2) * SC]], np.uint32),
            }
        )
    return in_maps


def run(trace=False, phases="ABCX", **inputs):
    """Run the distributed kernel; returns (output, exec_time_ns_or_None)."""
    in_maps = _prep_inputs(**inputs)
    if phases not in _compiled:
        _compiled[phases] = _build(phases)
    res = run_bass_kernel_spmd(
        _compiled[phases], in_maps, core_ids=list(range(NC_)), trace=trace
    )
    out = np.concatenate([res.results[c]["out"] for c in range(NC_)], axis=0)
    return out.reshape(1, N, D), res.exec_time_ns


def kernel(**inputs):
    out, _ = run(trace=False, **inputs)
    return out
